# revision 21
# baseline (speedup 1.0000x reference)
"""ExDETR matcher kernel for Trainium2 (8 NeuronCores, data-parallel over batch).

Problem: per image, compute a [N=900] matching cost
    C = 1*cost_class + 5*cost_bbox + 2*cost_giou + 10*cost_string
and return argmin over N.  Batch B=64 is sharded 8 images per core.

Per-core partition layout (128 partitions): images pair up into 32-partition
blocks (matmul PSUM outputs may only start at partition 0/32/64/96):
    row(i, j) = 32*(i//2) + 15*(i%2) + j        i = image 0..7, j = n-block 0..14
free g in [0,60), global query index n = j*60 + g.  Rows 30,31 of each block
are unused padding.

Per core:
  - string logits [75, 7200] per image -> exp (ScalarE) -> one-hot-column
    ones-matmuls over the 75 char partitions accumulate per-(n,l) sumexp into
    PSUM [128, 480] -> Ln -> reduce over l -> sum_l logsumexp [128, 60].
  - picked-char logits: 8 indirect row-gathers (element_offset=l makes the
    stride-8 positions of each gathered row exactly z[lps[l], n, l]) ->
    1.25-weighted matmuls over the 8 l-partitions -> PSUM [128, 60].
  - class cost: exp -> masked reduce (host one-hot mask) -> prob pick / sumexp.
  - bbox/giou: vertex min/max reduces + elementwise ops with per-partition
    target scalars (host-precomputed).
  - combine into NEGATED cost, per-partition top-1 via Max8/MaxIndex,
    output [128, 2] = (best -cost, candidate n); host picks the max over the
    15 rows of each image (a 15-way final select, ~nothing).
"""

import numpy as np

import concourse.bass as bass
import concourse.bacc as bacc
import concourse.mybir as mybir
import concourse.tile as tile
from concourse.bass_utils import run_bass_kernel_spmd

B, N, C = 64, 900, 16
NCHAR, L = 75, 8
NCORES = 8
NIMG = B // NCORES          # 8 images per core
NB = 15                     # n-blocks per image
G = N // NB                 # 60 queries per block
PT = 128                    # partitions
FLOG = N * L                # 7200 free elements per string-logit image tile

f32 = mybir.dt.float32
AF = mybir.ActivationFunctionType
ALU = mybir.AluOpType
AX = mybir.AxisListType

_PROGRAM = None


def _row0(i):
    return 32 * (i // 2) + 15 * (i % 2)


def _build_program():
    nc = bacc.Bacc(None, num_swdge_queues=4)

    slog = nc.declare_dram_parameter("slog", [NIMG * NCHAR, FLOG], f32, isOutput=False)
    plog = nc.declare_dram_parameter("plog", [PT, G * C], f32, isOutput=False)
    pbox = nc.declare_dram_parameter("pbox", [PT, G * 8], f32, isOutput=False)
    cmask = nc.declare_dram_parameter("cmask", [PT, C], f32, isOutput=False)
    taux = nc.declare_dram_parameter("taux", [PT, 8], f32, isOutput=False)
    goff = nc.declare_dram_parameter("goff", [NIMG, L], mybir.dt.int32, isOutput=False)
    # sliding-window one-hot column weights (f32r matmuls must sit at array
    # column 0, so every matmul is M=128 with zeros in the non-target columns):
    # image i slice for chunk j = wsum[:, 256*i + 128-j : 256*i + 256-j],
    # which has its all-ones column at out-row 32*(i//2) + 15*(i%2) + j.
    wsum = nc.declare_dram_parameter("wsum", [NCHAR, NIMG * 256], mybir.dt.float32r, isOutput=False)
    # pick: slice j = wpick[:, 128-j : 256-j]; col 32b+15p+j has 1.25 in rows
    # {8l + 2b + p} — one matmul per j covers all 8 images.
    wpick = nc.declare_dram_parameter("wpick", [64, 256], mybir.dt.float32r, isOutput=False)
    cand = nc.declare_dram_parameter("cand", [PT, 2], f32, isOutput=True)

    with tile.TileContext(nc) as tc:
        with (
            tc.tile_pool(name="small", bufs=1) as sm,
            tc.tile_pool(name="strt", bufs=3) as stp,
            tc.tile_pool(name="expt", bufs=2) as etp,
            tc.tile_pool(name="psum", bufs=1, space="PSUM") as pp,
        ):
            # ---- tiles for small inputs (DMAs staggered into the image
            # loop below: everything issued at t=0 round-robins on the same
            # SDMA engines and starves image 0's load) ----
            cm = sm.tile([PT, C], f32)
            ta = sm.tile([PT, 8], f32)
            go = sm.tile([NIMG, L], mybir.dt.int32)
            ws = sm.tile([NCHAR, NIMG * 256], mybir.dt.float32r)
            wp = sm.tile([64, 256], mybir.dt.float32r)
            pl = sm.tile([PT, G * C], f32)
            pb = sm.tile([PT, G * 8], f32)

            ps_str = pp.tile([PT, G * L], f32)   # sumexp per (n, l), one bank
            ps_pick = pp.tile([PT, G], f32)      # 1.25 * sum_l z_pick
            nc.vector.memset(ps_str[:], 0.0)
            nc.vector.memset(ps_pick[:], 0.0)

            # ---- string: picked-char gathers (early: they only need goff;
            # spread over the 4 SWDGE queues so the 64 single-partition rows
            # don't serialize through one SDMA engine) ----
            nc.sync.dma_start(out=go[:], in_=goff[:])
            pickL = sm.tile([NIMG * L, FLOG], mybir.dt.float32r)
            for l in range(L):
                inst = nc.gpsimd.indirect_dma_start(
                    out=pickL[8 * l:8 * (l + 1), :],
                    out_offset=None,
                    in_=slog[:].bitcast(mybir.dt.float32r),
                    in_offset=bass.IndirectOffsetOnAxis(ap=go[:, l:l + 1], axis=0),
                    element_offset=l,
                )
                q = l % 4
                inst.ins.queue = f"qPoolDynamic{q}" if q else "qPoolDynamic"

            # ---- string: per-image exp + char-sum matmuls (accumulating) ----
            for i in range(NIMG):
                st = stp.tile([NCHAR, FLOG], f32)
                nc.sync.dma_start(out=st[:], in_=slog[i * NCHAR:(i + 1) * NCHAR, :])
                if i == 0:
                    nc.sync.dma_start(out=ws[:], in_=wsum[:])
                elif i == 1:
                    nc.sync.dma_start(out=pl[:], in_=plog[:])
                    nc.sync.dma_start(out=pb[:], in_=pbox[:])
                    nc.sync.dma_start(out=cm[:], in_=cmask[:])
                    nc.sync.dma_start(out=ta[:], in_=taux[:])
                    nc.sync.dma_start(out=wp[:], in_=wpick[:])
                et = etp.tile([NCHAR, FLOG], mybir.dt.float32r)
                nc.scalar.activation(et[:], st[:], AF.Exp)
                for j in range(NB):
                    nc.tensor.matmul(
                        out=ps_str[:, :],
                        lhsT=ws[:, 256 * i + 128 - j:256 * i + 256 - j],
                        rhs=et[:][:, j * G * L:(j + 1) * G * L],
                        start=False,
                        stop=True,
                        skip_group_check=True,
                    )
            # picked elements of row l*8+i sit at stride-8 offsets 0,8,...
            rview = pickL[:].rearrange("p (n l) -> p l n", l=L)
            for j in range(NB):
                nc.tensor.matmul(
                    out=ps_pick[:, :],
                    lhsT=wp[:, 128 - j:256 - j],
                    rhs=rview[:, 0, j * G:(j + 1) * G],
                    start=False,
                    stop=True,
                    skip_group_check=True,
                )

            # ---- string cost: ln(sumexp), sum over l ----
            # Ln per 30-row block slice: padding rows hold sumexp=0 and
            # compute-engine APs must start at a 32-aligned partition.
            lnt = sm.tile([PT, G * L], f32)
            nc.vector.memset(lnt[:], 0.0)
            for blk in range(4):
                nc.scalar.activation(
                    lnt[32 * blk:32 * blk + 30, :],
                    ps_str[32 * blk:32 * blk + 30, :], AF.Ln)
            strsum = sm.tile([PT, G], f32)
            nc.vector.tensor_reduce(
                out=strsum[:],
                in_=lnt[:].rearrange("p (g l) -> p g l", l=L),
                axis=AX.X,
                op=ALU.add,
            )

            # ---- class cost: prob of target class ----
            ce = sm.tile([PT, G * C], f32)
            nc.scalar.activation(ce[:], pl[:], AF.Exp)
            se = sm.tile([PT, G], f32)
            nc.vector.tensor_reduce(
                out=se[:],
                in_=ce[:].rearrange("p (g c) -> p g c", c=C),
                axis=AX.X,
                op=ALU.add,
            )
            me = sm.tile([PT, G * C], f32)
            nc.vector.tensor_tensor(
                out=me[:].rearrange("p (g c) -> p g c", c=C),
                in0=ce[:].rearrange("p (g c) -> p g c", c=C),
                in1=cm[:].rearrange("p (o c) -> p o c", o=1).to_broadcast([PT, G, C]),
                op=ALU.mult,
            )
            pe_ = sm.tile([PT, G], f32)
            nc.vector.tensor_reduce(
                out=pe_[:],
                in_=me[:].rearrange("p (g c) -> p g c", c=C),
                axis=AX.X,
                op=ALU.add,
            )
            rse = sm.tile([PT, G], f32)
            nc.vector.reciprocal(rse[:], se[:])
            probt = sm.tile([PT, G], f32)
            nc.vector.tensor_tensor(out=probt[:], in0=pe_[:], in1=rse[:], op=ALU.mult)

            # ---- boxes: vertex min/max -> xyxy ----
            vb = pb[:].rearrange("p (g v x) -> p g x v", v=4, x=2)
            bmn = sm.tile([PT, G * 2], f32)
            nc.vector.tensor_reduce(
                out=bmn[:].rearrange("p (g x) -> p g x", x=2),
                in_=vb, axis=AX.X, op=ALU.min)
            bmx = sm.tile([PT, G * 2], f32)
            nc.vector.tensor_reduce(
                out=bmx[:].rearrange("p (g x) -> p g x", x=2),
                in_=vb, axis=AX.X, op=ALU.max)

            def coord(t, x):
                return t[:].rearrange("p (g x) -> p x g", x=2)[:, x, :]

            xm, ym = coord(bmn, 0), coord(bmn, 1)
            xM, yM = coord(bmx, 0), coord(bmx, 1)

            def tacol(k):
                # [PT, G] step-0 broadcast of a per-partition target scalar
                return ta[:, k:k + 1].rearrange("p (o c) -> p o c", o=1) \
                    .to_broadcast([PT, 1, G])[:, 0, :]

            tx1, ty1, tx2, ty2 = (tacol(k) for k in range(4))
            area_t = tacol(4)

            _sc = [0]

            def newt():
                _sc[0] += 1
                return sm.tile([PT, G], f32, name=f"scr{_sc[0]}")

            # ---- bbox L1 cost: 4 diffs interleaved, one |.|-sum reduce ----
            dt4 = sm.tile([PT, G * 4], f32)
            dv = dt4[:].rearrange("p (g k) -> p k g", k=4)
            for k, (src, tgt) in enumerate(
                    ((xm, tx1), (ym, ty1), (xM, tx2), (yM, ty2))):
                nc.vector.tensor_tensor(
                    out=dv[:, k, :], in0=src, in1=tgt, op=ALU.subtract)
            bbox = sm.tile([PT, G], f32)
            nc.vector.tensor_reduce(
                out=bbox[:], in_=dt4[:].rearrange("p (g k) -> p g k", k=4),
                axis=AX.X, op=ALU.add, apply_absolute_value=True)

            # ---- giou ----
            ltx, lty = newt(), newt()
            rbx, rby = newt(), newt()
            nc.vector.tensor_tensor(out=ltx[:], in0=xm, in1=tx1, op=ALU.max)
            nc.vector.tensor_tensor(out=lty[:], in0=ym, in1=ty1, op=ALU.max)
            nc.vector.tensor_tensor(out=rbx[:], in0=xM, in1=tx2, op=ALU.min)
            nc.vector.tensor_tensor(out=rby[:], in0=yM, in1=ty2, op=ALU.min)
            whx, why = newt(), newt()
            nc.vector.tensor_tensor(out=whx[:], in0=rbx[:], in1=ltx[:], op=ALU.subtract)
            nc.vector.tensor_scalar_max(whx[:], whx[:], 0.0)
            nc.vector.tensor_tensor(out=why[:], in0=rby[:], in1=lty[:], op=ALU.subtract)
            nc.vector.tensor_scalar_max(why[:], why[:], 0.0)
            inter = newt()
            nc.vector.tensor_tensor(out=inter[:], in0=whx[:], in1=why[:], op=ALU.mult)

            apx, apy, areap = newt(), newt(), newt()
            nc.vector.tensor_tensor(out=apx[:], in0=xM, in1=xm, op=ALU.subtract)
            nc.vector.tensor_tensor(out=apy[:], in0=yM, in1=ym, op=ALU.subtract)
            nc.vector.tensor_tensor(out=areap[:], in0=apx[:], in1=apy[:], op=ALU.mult)
            union = newt()
            nc.vector.tensor_tensor(out=union[:], in0=areap[:], in1=area_t, op=ALU.add)
            nc.vector.tensor_tensor(out=union[:], in0=union[:], in1=inter[:], op=ALU.subtract)
            runion = newt()
            nc.vector.reciprocal(runion[:], union[:])
            iou = newt()
            nc.vector.tensor_tensor(out=iou[:], in0=inter[:], in1=runion[:], op=ALU.mult)

            # enclosing box (no clip needed: max-corner >= min-corner always)
            lcx, lcy, rcx, rcy = newt(), newt(), newt(), newt()
            nc.vector.tensor_tensor(out=lcx[:], in0=xm, in1=tx1, op=ALU.min)
            nc.vector.tensor_tensor(out=lcy[:], in0=ym, in1=ty1, op=ALU.min)
            nc.vector.tensor_tensor(out=rcx[:], in0=xM, in1=tx2, op=ALU.max)
            nc.vector.tensor_tensor(out=rcy[:], in0=yM, in1=ty2, op=ALU.max)
            wcx, wcy, areac = newt(), newt(), newt()
            nc.vector.tensor_tensor(out=wcx[:], in0=rcx[:], in1=lcx[:], op=ALU.subtract)
            nc.vector.tensor_tensor(out=wcy[:], in0=rcy[:], in1=lcy[:], op=ALU.subtract)
            nc.vector.tensor_tensor(out=areac[:], in0=wcx[:], in1=wcy[:], op=ALU.mult)
            rareac = newt()
            nc.vector.reciprocal(rareac[:], areac[:])
            # giou + 1 == iou + union/area_c (constant +1 dropped; argmin-invariant)
            giou1 = newt()
            nc.vector.tensor_tensor(out=giou1[:], in0=union[:], in1=rareac[:], op=ALU.mult)
            nc.vector.tensor_tensor(out=giou1[:], in0=giou1[:], in1=iou[:], op=ALU.add)

            # ---- combine: negated cost ----
            # Cneg = prob - 5*bbox + 2*(giou+1) - 1.25*sum_l LSE + ps_pick  (+ const)
            t1 = newt()
            nc.vector.scalar_tensor_tensor(
                out=t1[:], in0=bbox[:], scalar=-5.0, in1=probt[:],
                op0=ALU.mult, op1=ALU.add)
            t2 = newt()
            nc.vector.scalar_tensor_tensor(
                out=t2[:], in0=giou1[:], scalar=2.0, in1=t1[:],
                op0=ALU.mult, op1=ALU.add)
            t3 = newt()
            nc.vector.scalar_tensor_tensor(
                out=t3[:], in0=strsum[:], scalar=-1.25, in1=t2[:],
                op0=ALU.mult, op1=ALU.add)
            cneg = sm.tile([PT, G], f32)
            nc.vector.tensor_tensor(out=cneg[:], in0=t3[:], in1=ps_pick[:], op=ALU.add)

            # ---- per-partition top-1 ----
            mx8v = sm.tile([PT, 8], f32)
            nc.vector.max(mx8v[:], cneg[:])
            mx8i = sm.tile([PT, 8], mybir.dt.uint32)
            nc.vector.max_index(mx8i[:], mx8v[:], cneg[:])
            idxf = sm.tile([PT, 1], f32)
            nc.vector.tensor_copy(idxf[:], mx8i[:, 0:1])
            packed = sm.tile([PT, 2], f32)
            nc.vector.tensor_copy(packed[:, 0:1], mx8v[:, 0:1])
            nc.vector.tensor_tensor(out=packed[:, 1:2], in0=idxf[:], in1=ta[:, 5:6], op=ALU.add)

            nc.sync.dma_start(out=cand[:], in_=packed[:])

    nc.finalize()
    return nc


def _get_program():
    global _PROGRAM
    if _PROGRAM is None:
        _PROGRAM = _build_program()
    return _PROGRAM


def _weight_consts():
    wsum = np.zeros((NCHAR, NIMG * 256), dtype=np.float32)
    for i in range(NIMG):
        wsum[:, 256 * i + 128 + _row0(i)] = 1.0
    wpick = np.zeros((64, 256), dtype=np.float32)
    for blk in range(4):
        for p in range(2):
            for l in range(L):
                wpick[8 * l + 2 * blk + p, 128 + 32 * blk + 15 * p] = 10.0 / L
    return wsum, wpick


_WSUM, _WPICK = _weight_consts()


def _prep_core_inputs(pred_logits, pred_boxes, pred_string_logits,
                      tgt_bboxes, plate_type, lps, core):
    i0 = core * NIMG
    sl = np.ascontiguousarray(
        pred_string_logits[i0:i0 + NIMG].reshape(NIMG * NCHAR, FLOG), dtype=np.float32)

    plog = np.zeros((PT, G * C), dtype=np.float32)
    pbox = np.full((PT, G * 8), 0.5, dtype=np.float32)
    cmask = np.zeros((PT, C), dtype=np.float32)
    cmask[:, 0] = 1.0
    taux = np.zeros((PT, 8), dtype=np.float32)
    taux[:, 2:5] = 1.0

    pl_src = pred_logits[i0:i0 + NIMG].reshape(NIMG, NB, G * C)
    pb_src = pred_boxes[i0:i0 + NIMG].reshape(NIMG, NB, G * 8)
    for i in range(NIMG):
        b = i0 + i
        r0 = _row0(i)
        rows = slice(r0, r0 + NB)
        plog[rows] = pl_src[i]
        pbox[rows] = pb_src[i]
        t = tgt_bboxes[b].astype(np.float32)
        cmask[rows] = 0.0
        cmask[rows, int(plate_type[b])] = 1.0
        taux[rows, 0:4] = t
        taux[rows, 4] = (t[2] - t[0]) * (t[3] - t[1])
        taux[rows, 5] = np.arange(NB, dtype=np.float32) * G

    goff = np.zeros((NIMG, L), dtype=np.int32)
    for i in range(NIMG):
        goff[i, :] = i * NCHAR + lps[i0 + i].astype(np.int64)

    return {"slog": sl, "plog": plog, "pbox": pbox, "cmask": cmask,
            "taux": taux, "goff": goff, "wsum": _WSUM, "wpick": _WPICK}


def _finish(cand_list):
    """cand_list: per-core [128, 2] arrays -> [64, 1] int32 argmin indices."""
    out = np.zeros((B, 1), dtype=np.int32)
    for core, cd in enumerate(cand_list):
        for i in range(NIMG):
            r0 = _row0(i)
            v = cd[r0:r0 + NB, 0]
            n = cd[r0:r0 + NB, 1]
            best = v.max()
            # ties across blocks: pick smallest n (matches argmin first-index)
            out[core * NIMG + i, 0] = np.int32(n[v == best].min())
    return out


def kernel(pred_logits, pred_boxes, pred_string_logits, tgt_bboxes,
           plate_type, lps, **run_kwargs):
    nc = _get_program()
    in_maps = [
        _prep_core_inputs(pred_logits, pred_boxes, pred_string_logits,
                          tgt_bboxes, plate_type, lps, core)
        for core in range(NCORES)
    ]
    res = run_bass_kernel_spmd(nc, in_maps, list(range(NCORES)), **run_kwargs)
    cands = [res.results[c]["cand"] for c in range(NCORES)]
    out = _finish(cands)
    kernel.last_results = res
    return out


# revision 22
# speedup vs baseline: 1.2232x; 1.2232x over previous
"""ExDETR matcher kernel for Trainium2 (8 NeuronCores, data-parallel over batch).

Problem: per image, compute a [N=900] matching cost
    C = 1*cost_class + 5*cost_bbox + 2*cost_giou + 10*cost_string
and return argmin over N.  Batch B=64 is sharded 8 images per core.

Per-core partition layout (128 partitions): images pair up into 32-partition
blocks (matmul PSUM outputs may only start at partition 0/32/64/96):
    row(i, j) = 32*(i//2) + 15*(i%2) + j        i = image 0..7, j = n-block 0..14
free g in [0,60), global query index n = j*60 + g.  Rows 30,31 of each block
are unused padding.

Per core:
  - string logits [75, 7200] per image -> exp (ScalarE) -> one-hot-column
    ones-matmuls over the 75 char partitions accumulate per-(n,l) sumexp into
    PSUM [128, 480] -> Ln -> reduce over l -> sum_l logsumexp [128, 60].
  - picked-char logits: 8 indirect row-gathers (element_offset=l makes the
    stride-8 positions of each gathered row exactly z[lps[l], n, l]) ->
    1.25-weighted matmuls over the 8 l-partitions -> PSUM [128, 60].
  - class cost: exp -> masked reduce (host one-hot mask) -> prob pick / sumexp.
  - bbox/giou: vertex min/max reduces + elementwise ops with per-partition
    target scalars (host-precomputed).
  - combine into NEGATED cost, per-partition top-1 via Max8/MaxIndex,
    output [128, 2] = (best -cost, candidate n); host picks the max over the
    15 rows of each image (a 15-way final select, ~nothing).
"""

import numpy as np

import concourse.bass as bass
import concourse.bacc as bacc
import concourse.mybir as mybir
import concourse.tile as tile
from concourse.bass_utils import run_bass_kernel_spmd

B, N, C = 64, 900, 16
NCHAR, L = 75, 8
NCORES = 8
NIMG = B // NCORES          # 8 images per core
NB = 15                     # n-blocks per image
G = N // NB                 # 60 queries per block
PT = 128                    # partitions
FLOG = N * L                # 7200 free elements per string-logit image tile

f32 = mybir.dt.float32
AF = mybir.ActivationFunctionType
ALU = mybir.AluOpType
AX = mybir.AxisListType

_PROGRAM = None


def _row0(i):
    return 32 * (i // 2) + 15 * (i % 2)


def _build_program():
    nc = bacc.Bacc(None, num_swdge_queues=4)

    slog = nc.declare_dram_parameter("slog", [NIMG * NCHAR, FLOG], f32, isOutput=False)
    plog = nc.declare_dram_parameter("plog", [PT, G * C], f32, isOutput=False)
    pbox = nc.declare_dram_parameter("pbox", [PT, G * 8], f32, isOutput=False)
    cmask = nc.declare_dram_parameter("cmask", [PT, C], f32, isOutput=False)
    taux = nc.declare_dram_parameter("taux", [PT, 8], f32, isOutput=False)
    # l-major transposed copy of the string logits: row (i*75+c)*8+l is the
    # contiguous [900] vector z[c, :, l] of image i — makes the picked-char
    # gather 64 small contiguous reads instead of 1.8MB of full rows.
    slogt = nc.declare_dram_parameter("slogt", [NIMG * NCHAR * L, N], f32, isOutput=False)
    goff = nc.declare_dram_parameter("goff", [NIMG * L, 1], mybir.dt.int32, isOutput=False)
    # sliding-window one-hot column weights (f32r matmuls must sit at array
    # column 0, so every matmul is M=128 with zeros in the non-target columns):
    # image i slice for chunk j = wsum[:, 256*i + 128-j : 256*i + 256-j],
    # which has its all-ones column at out-row 32*(i//2) + 15*(i%2) + j.
    wsum = nc.declare_dram_parameter("wsum", [NCHAR, NIMG * 256], mybir.dt.float32r, isOutput=False)
    # pick: slice j = wpick[:, 128-j : 256-j]; col 32b+15p+j has 1.25 in rows
    # {8l + 2b + p} — one matmul per j covers all 8 images.
    wpick = nc.declare_dram_parameter("wpick", [64, 256], mybir.dt.float32r, isOutput=False)
    cand = nc.declare_dram_parameter("cand", [PT, 2], f32, isOutput=True)

    with tile.TileContext(nc) as tc:
        with (
            tc.tile_pool(name="small", bufs=1) as sm,
            tc.tile_pool(name="strt", bufs=3) as stp,
            tc.tile_pool(name="expt", bufs=2) as etp,
            tc.tile_pool(name="psum", bufs=1, space="PSUM") as pp,
        ):
            # ---- tiles for small inputs (DMAs staggered into the image
            # loop below: everything issued at t=0 round-robins on the same
            # SDMA engines and starves image 0's load) ----
            cm = sm.tile([PT, C], f32)
            ta = sm.tile([PT, 8], f32)
            go = sm.tile([NIMG * L, 1], mybir.dt.int32)
            ws = sm.tile([NCHAR, NIMG * 256], mybir.dt.float32r)
            wp = sm.tile([64, 256], mybir.dt.float32r)
            pl = sm.tile([PT, G * C], f32)
            pb = sm.tile([PT, G * 8], f32)

            ps_str = pp.tile([PT, G * L], f32)   # sumexp per (n, l), one bank
            ps_pick = pp.tile([PT, G], f32)      # 1.25 * sum_l z_pick
            nc.vector.memset(ps_str[:], 0.0)
            nc.vector.memset(ps_pick[:], 0.0)

            # ---- string: picked-char gather (early; 64 contiguous [900]
            # rows from the l-transposed copy, row p = (l, i) = 8l + i) ----
            nc.sync.dma_start(out=go[:], in_=goff[:])
            pickL = sm.tile([NIMG * L, N], mybir.dt.float32r)
            nc.gpsimd.indirect_dma_start(
                out=pickL[:],
                out_offset=None,
                in_=slogt[:].bitcast(mybir.dt.float32r),
                in_offset=bass.IndirectOffsetOnAxis(ap=go[:, 0:1], axis=0),
            )

            # ---- string: per-image exp + char-sum matmuls (accumulating) ----
            for i in range(NIMG):
                st = stp.tile([NCHAR, FLOG], f32)
                nc.sync.dma_start(out=st[:], in_=slog[i * NCHAR:(i + 1) * NCHAR, :])
                if i == 0:
                    nc.sync.dma_start(out=ws[:], in_=wsum[:])
                    nc.sync.dma_start(out=wp[:], in_=wpick[:])
                elif i == 1:
                    nc.sync.dma_start(out=pl[:], in_=plog[:])
                    nc.sync.dma_start(out=pb[:], in_=pbox[:])
                    nc.sync.dma_start(out=cm[:], in_=cmask[:])
                    nc.sync.dma_start(out=ta[:], in_=taux[:])
                et = etp.tile([NCHAR, FLOG], mybir.dt.float32r)
                nc.scalar.activation(et[:], st[:], AF.Exp)
                for j in range(NB):
                    nc.tensor.matmul(
                        out=ps_str[:, :],
                        lhsT=ws[:, 256 * i + 128 - j:256 * i + 256 - j],
                        rhs=et[:][:, j * G * L:(j + 1) * G * L],
                        start=False,
                        stop=True,
                        skip_group_check=True,
                    )
            for j in range(NB):
                nc.tensor.matmul(
                    out=ps_pick[:, :],
                    lhsT=wp[:, 128 - j:256 - j],
                    rhs=pickL[:, j * G:(j + 1) * G],
                    start=False,
                    stop=True,
                    skip_group_check=True,
                )

            # ---- string cost: ln(sumexp), sum over l ----
            # Ln per 30-row block slice: padding rows hold sumexp=0 and
            # compute-engine APs must start at a 32-aligned partition.
            lnt = sm.tile([PT, G * L], f32)
            nc.vector.memset(lnt[:], 0.0)
            for blk in range(4):
                nc.scalar.activation(
                    lnt[32 * blk:32 * blk + 30, :],
                    ps_str[32 * blk:32 * blk + 30, :], AF.Ln)
            strsum = sm.tile([PT, G], f32)
            nc.vector.tensor_reduce(
                out=strsum[:],
                in_=lnt[:].rearrange("p (g l) -> p g l", l=L),
                axis=AX.X,
                op=ALU.add,
            )

            # ---- class cost: prob of target class ----
            ce = sm.tile([PT, G * C], f32)
            nc.scalar.activation(ce[:], pl[:], AF.Exp)
            se = sm.tile([PT, G], f32)
            nc.vector.tensor_reduce(
                out=se[:],
                in_=ce[:].rearrange("p (g c) -> p g c", c=C),
                axis=AX.X,
                op=ALU.add,
            )
            me = sm.tile([PT, G * C], f32)
            nc.vector.tensor_tensor(
                out=me[:].rearrange("p (g c) -> p g c", c=C),
                in0=ce[:].rearrange("p (g c) -> p g c", c=C),
                in1=cm[:].rearrange("p (o c) -> p o c", o=1).to_broadcast([PT, G, C]),
                op=ALU.mult,
            )
            pe_ = sm.tile([PT, G], f32)
            nc.vector.tensor_reduce(
                out=pe_[:],
                in_=me[:].rearrange("p (g c) -> p g c", c=C),
                axis=AX.X,
                op=ALU.add,
            )
            rse = sm.tile([PT, G], f32)
            nc.vector.reciprocal(rse[:], se[:])
            probt = sm.tile([PT, G], f32)
            nc.vector.tensor_tensor(out=probt[:], in0=pe_[:], in1=rse[:], op=ALU.mult)

            # ---- boxes: vertex min/max -> xyxy ----
            vb = pb[:].rearrange("p (g v x) -> p g x v", v=4, x=2)
            bmn = sm.tile([PT, G * 2], f32)
            nc.vector.tensor_reduce(
                out=bmn[:].rearrange("p (g x) -> p g x", x=2),
                in_=vb, axis=AX.X, op=ALU.min)
            bmx = sm.tile([PT, G * 2], f32)
            nc.vector.tensor_reduce(
                out=bmx[:].rearrange("p (g x) -> p g x", x=2),
                in_=vb, axis=AX.X, op=ALU.max)

            def coord(t, x):
                return t[:].rearrange("p (g x) -> p x g", x=2)[:, x, :]

            xm, ym = coord(bmn, 0), coord(bmn, 1)
            xM, yM = coord(bmx, 0), coord(bmx, 1)

            def tacol(k):
                # [PT, G] step-0 broadcast of a per-partition target scalar
                return ta[:, k:k + 1].rearrange("p (o c) -> p o c", o=1) \
                    .to_broadcast([PT, 1, G])[:, 0, :]

            tx1, ty1, tx2, ty2 = (tacol(k) for k in range(4))
            area_t = tacol(4)

            _sc = [0]

            def newt():
                _sc[0] += 1
                return sm.tile([PT, G], f32, name=f"scr{_sc[0]}")

            # ---- bbox L1 cost: 4 diffs interleaved, one |.|-sum reduce ----
            dt4 = sm.tile([PT, G * 4], f32)
            dv = dt4[:].rearrange("p (g k) -> p k g", k=4)
            for k, (src, tgt) in enumerate(
                    ((xm, tx1), (ym, ty1), (xM, tx2), (yM, ty2))):
                nc.vector.tensor_tensor(
                    out=dv[:, k, :], in0=src, in1=tgt, op=ALU.subtract)
            bbox = sm.tile([PT, G], f32)
            nc.vector.tensor_reduce(
                out=bbox[:], in_=dt4[:].rearrange("p (g k) -> p g k", k=4),
                axis=AX.X, op=ALU.add, apply_absolute_value=True)

            # ---- giou ----
            ltx, lty = newt(), newt()
            rbx, rby = newt(), newt()
            nc.vector.tensor_tensor(out=ltx[:], in0=xm, in1=tx1, op=ALU.max)
            nc.vector.tensor_tensor(out=lty[:], in0=ym, in1=ty1, op=ALU.max)
            nc.vector.tensor_tensor(out=rbx[:], in0=xM, in1=tx2, op=ALU.min)
            nc.vector.tensor_tensor(out=rby[:], in0=yM, in1=ty2, op=ALU.min)
            whx, why = newt(), newt()
            nc.vector.tensor_tensor(out=whx[:], in0=rbx[:], in1=ltx[:], op=ALU.subtract)
            nc.vector.tensor_scalar_max(whx[:], whx[:], 0.0)
            nc.vector.tensor_tensor(out=why[:], in0=rby[:], in1=lty[:], op=ALU.subtract)
            nc.vector.tensor_scalar_max(why[:], why[:], 0.0)
            inter = newt()
            nc.vector.tensor_tensor(out=inter[:], in0=whx[:], in1=why[:], op=ALU.mult)

            apx, apy, areap = newt(), newt(), newt()
            nc.vector.tensor_tensor(out=apx[:], in0=xM, in1=xm, op=ALU.subtract)
            nc.vector.tensor_tensor(out=apy[:], in0=yM, in1=ym, op=ALU.subtract)
            nc.vector.tensor_tensor(out=areap[:], in0=apx[:], in1=apy[:], op=ALU.mult)
            union = newt()
            nc.vector.tensor_tensor(out=union[:], in0=areap[:], in1=area_t, op=ALU.add)
            nc.vector.tensor_tensor(out=union[:], in0=union[:], in1=inter[:], op=ALU.subtract)
            runion = newt()
            nc.vector.reciprocal(runion[:], union[:])
            iou = newt()
            nc.vector.tensor_tensor(out=iou[:], in0=inter[:], in1=runion[:], op=ALU.mult)

            # enclosing box (no clip needed: max-corner >= min-corner always)
            lcx, lcy, rcx, rcy = newt(), newt(), newt(), newt()
            nc.vector.tensor_tensor(out=lcx[:], in0=xm, in1=tx1, op=ALU.min)
            nc.vector.tensor_tensor(out=lcy[:], in0=ym, in1=ty1, op=ALU.min)
            nc.vector.tensor_tensor(out=rcx[:], in0=xM, in1=tx2, op=ALU.max)
            nc.vector.tensor_tensor(out=rcy[:], in0=yM, in1=ty2, op=ALU.max)
            wcx, wcy, areac = newt(), newt(), newt()
            nc.vector.tensor_tensor(out=wcx[:], in0=rcx[:], in1=lcx[:], op=ALU.subtract)
            nc.vector.tensor_tensor(out=wcy[:], in0=rcy[:], in1=lcy[:], op=ALU.subtract)
            nc.vector.tensor_tensor(out=areac[:], in0=wcx[:], in1=wcy[:], op=ALU.mult)
            rareac = newt()
            nc.vector.reciprocal(rareac[:], areac[:])
            # giou + 1 == iou + union/area_c (constant +1 dropped; argmin-invariant)
            giou1 = newt()
            nc.vector.tensor_tensor(out=giou1[:], in0=union[:], in1=rareac[:], op=ALU.mult)
            nc.vector.tensor_tensor(out=giou1[:], in0=giou1[:], in1=iou[:], op=ALU.add)

            # ---- combine: negated cost ----
            # Cneg = prob - 5*bbox + 2*(giou+1) - 1.25*sum_l LSE + ps_pick  (+ const)
            t1 = newt()
            nc.vector.scalar_tensor_tensor(
                out=t1[:], in0=bbox[:], scalar=-5.0, in1=probt[:],
                op0=ALU.mult, op1=ALU.add)
            t2 = newt()
            nc.vector.scalar_tensor_tensor(
                out=t2[:], in0=giou1[:], scalar=2.0, in1=t1[:],
                op0=ALU.mult, op1=ALU.add)
            t3 = newt()
            nc.vector.scalar_tensor_tensor(
                out=t3[:], in0=strsum[:], scalar=-1.25, in1=t2[:],
                op0=ALU.mult, op1=ALU.add)
            cneg = sm.tile([PT, G], f32)
            nc.vector.tensor_tensor(out=cneg[:], in0=t3[:], in1=ps_pick[:], op=ALU.add)

            # ---- per-partition top-1 ----
            mx8v = sm.tile([PT, 8], f32)
            nc.vector.max(mx8v[:], cneg[:])
            mx8i = sm.tile([PT, 8], mybir.dt.uint32)
            nc.vector.max_index(mx8i[:], mx8v[:], cneg[:])
            idxf = sm.tile([PT, 1], f32)
            nc.vector.tensor_copy(idxf[:], mx8i[:, 0:1])
            packed = sm.tile([PT, 2], f32)
            nc.vector.tensor_copy(packed[:, 0:1], mx8v[:, 0:1])
            nc.vector.tensor_tensor(out=packed[:, 1:2], in0=idxf[:], in1=ta[:, 5:6], op=ALU.add)

            nc.sync.dma_start(out=cand[:], in_=packed[:])

    nc.finalize()
    return nc


def _get_program():
    global _PROGRAM
    if _PROGRAM is None:
        _PROGRAM = _build_program()
    return _PROGRAM


def _weight_consts():
    wsum = np.zeros((NCHAR, NIMG * 256), dtype=np.float32)
    for i in range(NIMG):
        wsum[:, 256 * i + 128 + _row0(i)] = 1.0
    wpick = np.zeros((64, 256), dtype=np.float32)
    for blk in range(4):
        for p in range(2):
            for l in range(L):
                wpick[8 * l + 2 * blk + p, 128 + 32 * blk + 15 * p] = 10.0 / L
    return wsum, wpick


_WSUM, _WPICK = _weight_consts()


def _prep_core_inputs(pred_logits, pred_boxes, pred_string_logits,
                      tgt_bboxes, plate_type, lps, core):
    i0 = core * NIMG
    sl = np.ascontiguousarray(
        pred_string_logits[i0:i0 + NIMG].reshape(NIMG * NCHAR, FLOG), dtype=np.float32)
    slt = np.ascontiguousarray(
        pred_string_logits[i0:i0 + NIMG].transpose(0, 1, 3, 2)
        .reshape(NIMG * NCHAR * L, N), dtype=np.float32)

    plog = np.zeros((PT, G * C), dtype=np.float32)
    pbox = np.full((PT, G * 8), 0.5, dtype=np.float32)
    cmask = np.zeros((PT, C), dtype=np.float32)
    cmask[:, 0] = 1.0
    taux = np.zeros((PT, 8), dtype=np.float32)
    taux[:, 2:5] = 1.0

    pl_src = pred_logits[i0:i0 + NIMG].reshape(NIMG, NB, G * C)
    pb_src = pred_boxes[i0:i0 + NIMG].reshape(NIMG, NB, G * 8)
    for i in range(NIMG):
        b = i0 + i
        r0 = _row0(i)
        rows = slice(r0, r0 + NB)
        plog[rows] = pl_src[i]
        pbox[rows] = pb_src[i]
        t = tgt_bboxes[b].astype(np.float32)
        cmask[rows] = 0.0
        cmask[rows, int(plate_type[b])] = 1.0
        taux[rows, 0:4] = t
        taux[rows, 4] = (t[2] - t[0]) * (t[3] - t[1])
        taux[rows, 5] = np.arange(NB, dtype=np.float32) * G

    # row p = 8l + i -> transposed-copy row index (i*75 + c^i_l)*8 + l
    goff = np.zeros((NIMG * L, 1), dtype=np.int32)
    for l in range(L):
        for i in range(NIMG):
            goff[8 * l + i, 0] = (i * NCHAR + int(lps[i0 + i, l])) * L + l

    return {"slog": sl, "slogt": slt, "plog": plog, "pbox": pbox,
            "cmask": cmask, "taux": taux, "goff": goff,
            "wsum": _WSUM, "wpick": _WPICK}


def _finish(cand_list):
    """cand_list: per-core [128, 2] arrays -> [64, 1] int32 argmin indices."""
    out = np.zeros((B, 1), dtype=np.int32)
    for core, cd in enumerate(cand_list):
        for i in range(NIMG):
            r0 = _row0(i)
            v = cd[r0:r0 + NB, 0]
            n = cd[r0:r0 + NB, 1]
            best = v.max()
            # ties across blocks: pick smallest n (matches argmin first-index)
            out[core * NIMG + i, 0] = np.int32(n[v == best].min())
    return out


def kernel(pred_logits, pred_boxes, pred_string_logits, tgt_bboxes,
           plate_type, lps, **run_kwargs):
    nc = _get_program()
    in_maps = [
        _prep_core_inputs(pred_logits, pred_boxes, pred_string_logits,
                          tgt_bboxes, plate_type, lps, core)
        for core in range(NCORES)
    ]
    res = run_bass_kernel_spmd(nc, in_maps, list(range(NCORES)), **run_kwargs)
    cands = [res.results[c]["cand"] for c in range(NCORES)]
    out = _finish(cands)
    kernel.last_results = res
    return out
